# revision 1
# baseline (speedup 1.0000x reference)
"""Chebyshev ASPIRE layer on 8 Trainium2 NeuronCores.

Strategy: precompute the dense Gram matrix B = X^T X [20000 x 20000] on host
(one sparse matmul), pad items to 20480. Each core owns a 2560-row slab of
the output/state; B's matching 2560-column slab is resident in DRAM as bf16
and streamed through SBUF each pass. The full T_k vector block [20480 x 128]
is replicated in SBUF (bf16); each Chebyshev pass does the slab matvec on
PE (128x128 tiles, fp32 PSUM accumulation over 160 k-chunks), the recurrence
update on DVE in fp32, then an AllGather of the 8 bf16 slabs rebuilds the
full T_{k+1} on every core.
"""
import numpy as np
import scipy.sparse as sp
import ml_dtypes

import concourse.bass as bass
import concourse.mybir as mybir
import concourse.tile as tile
from concourse import bacc, bass_utils

N_CORES = 8
BATCH = 128
N_K = 160          # item chunks of 128 -> ITEMS_PAD = 20480
N_M = 20           # slab chunks of 128 -> SLAB = 2560
ITEMS_PAD = N_K * 128
SLAB = N_M * 128
F32 = mybir.dt.float32
BF16 = mybir.dt.bfloat16

_cache = {}


def _build(coeffs, t_mid, t_half):
    coeffs = [float(x) for x in np.asarray(coeffs)]
    t_mid = float(t_mid)
    t_half = float(t_half)
    n_passes = len(coeffs) - 1
    inv_th = 1.0 / t_half
    b_rec = 2.0 * t_mid / t_half      # Tn = (2/th) y - b_rec*Tc - Tp
    a_rec = 2.0 / t_half

    nc = bacc.Bacc("TRN2", target_bir_lowering=False, debug=False,
                   num_devices=N_CORES)

    bsw = nc.dram_tensor("bsw", [128, N_M, N_K, 128], BF16,
                         kind="ExternalInput")
    tfull0 = nc.dram_tensor("tfull0", [128, N_K, BATCH], BF16,
                            kind="ExternalInput")
    t0slab = nc.dram_tensor("t0slab", [128, N_M, BATCH], F32,
                            kind="ExternalInput")
    acc_out = nc.dram_tensor("acc_out", [128, N_M, BATCH], F32,
                             kind="ExternalOutput")

    with tile.TileContext(nc) as tc:
        with tc.tile_pool(name="persist", bufs=1) as persist, \
             tc.tile_pool(name="bblk", bufs=2) as bpool, \
             tc.tile_pool(name="scratch", bufs=4) as scratch, \
             tc.tile_pool(name="dram", bufs=1, space="DRAM") as dram, \
             tc.tile_pool(name="psum", bufs=4, space="PSUM") as psum:

            # shared DRAM tensors may only be written once -> one pair per pass
            cc_ins = [dram.tile([128, N_M, BATCH], BF16, name=f"cc_in_{p}")
                      for p in range(n_passes - 1)]
            cc_outs = [dram.tile([N_CORES, 128, N_M, BATCH], BF16,
                                 addr_space="Shared", name=f"cc_out_{p}")
                       for p in range(n_passes - 1)]

            tfull = persist.tile([128, N_K, BATCH], BF16, tag="tfull")
            tprev = persist.tile([128, N_M, BATCH], F32, tag="tprev")
            tcurr = persist.tile([128, N_M, BATCH], F32, tag="tcurr")
            acc = persist.tile([128, N_M, BATCH], F32, tag="acc")
            tn16 = persist.tile([128, N_M, BATCH], BF16, tag="tn16")

            nc.sync.dma_start(tfull[:], tfull0.ap())
            nc.sync.dma_start(tprev[:], t0slab.ap())

            Tp, Tc = tprev, tcurr

            for p in range(n_passes):
                k_idx = p + 1          # computing T_{k_idx}
                last = (p == n_passes - 1)
                for m in range(N_M):
                    bblk = bpool.tile([128, N_K, 128], BF16, tag="bblk")
                    nc.sync.dma_start(bblk[:], bsw.ap()[:, m])
                    ps = psum.tile([128, BATCH], F32, tag="ps")
                    for k in range(N_K):
                        nc.tensor.matmul(
                            ps[:], bblk[:, k], tfull[:, k],
                            start=(k == 0), stop=(k == N_K - 1),
                        )
                    u = scratch.tile([128, BATCH], F32, tag="u")
                    if p == 0:
                        # T1 = inv_th*y - (tm/th)*T0 ; T0 lives in Tp
                        nc.vector.tensor_scalar_mul(u[:], Tp[:, m], t_mid * inv_th)
                        nc.vector.scalar_tensor_tensor(
                            Tc[:, m], ps[:], inv_th, u[:],
                            mybir.AluOpType.mult, mybir.AluOpType.subtract)
                        nc.vector.tensor_scalar_mul(u[:], Tp[:, m], coeffs[0])
                        nc.vector.scalar_tensor_tensor(
                            acc[:, m], Tc[:, m], coeffs[1], u[:],
                            mybir.AluOpType.mult, mybir.AluOpType.add)
                        if not last:
                            nc.vector.tensor_copy(tn16[:, m], Tc[:, m])
                    else:
                        # u = a_rec*y - Tp ; Tn = -b_rec*Tc + u  (into Tp buf)
                        nc.vector.scalar_tensor_tensor(
                            u[:], ps[:], a_rec, Tp[:, m],
                            mybir.AluOpType.mult, mybir.AluOpType.subtract)
                        nc.vector.scalar_tensor_tensor(
                            Tp[:, m], Tc[:, m], -b_rec, u[:],
                            mybir.AluOpType.mult, mybir.AluOpType.add)
                        nc.vector.scalar_tensor_tensor(
                            acc[:, m], Tp[:, m], coeffs[k_idx], acc[:, m],
                            mybir.AluOpType.mult, mybir.AluOpType.add)
                        if not last:
                            nc.vector.tensor_copy(tn16[:, m], Tp[:, m])
                if p > 0:
                    Tp, Tc = Tc, Tp
                if not last:
                    cc_in, cc_out = cc_ins[p], cc_outs[p]
                    nc.sync.dma_start(cc_in[:], tn16[:])
                    nc.gpsimd.collective_compute(
                        "AllGather", mybir.AluOpType.bypass,
                        replica_groups=[list(range(N_CORES))],
                        ins=[cc_in.opt()], outs=[cc_out.opt()],
                    )
                    # cc_out[c, p, m, b] -> tfull[p, c*N_M + m, b]
                    src = cc_out.rearrange("c p m b -> p c m b")
                    dst = tfull[:].rearrange("p (c m) b -> p c m b", c=N_CORES)
                    nc.sync.dma_start(dst, src)

            nc.sync.dma_start(acc_out.ap(), acc[:])

    nc.compile()
    return nc


def _prep_inputs(B, X_batch):
    n_items = B.shape[0]
    B16 = np.zeros((ITEMS_PAD, ITEMS_PAD), dtype=ml_dtypes.bfloat16)
    B16[:n_items, :n_items] = B.astype(ml_dtypes.bfloat16)

    T0 = np.zeros((ITEMS_PAD, BATCH), dtype=np.float32)
    T0[:n_items] = X_batch.T.astype(np.float32)
    tfull0 = np.ascontiguousarray(
        T0.reshape(N_K, 128, BATCH).transpose(1, 0, 2)).astype(ml_dtypes.bfloat16)

    in_maps = []
    for c in range(N_CORES):
        slab = B16[:, c * SLAB:(c + 1) * SLAB]
        bsw = np.ascontiguousarray(
            slab.reshape(N_K, 128, N_M, 128).transpose(1, 2, 0, 3))
        t0slab = np.ascontiguousarray(
            T0[c * SLAB:(c + 1) * SLAB].reshape(N_M, 128, BATCH)
            .transpose(1, 0, 2))
        in_maps.append({"bsw": bsw, "tfull0": tfull0, "t0slab": t0slab})
    return in_maps


def kernel(X_batch, vals, coeffs, t_mid, t_half, rows, cols, n_users):
    X_batch = np.asarray(X_batch)
    vals = np.asarray(vals)
    coeffs = np.asarray(coeffs)
    rows = np.asarray(rows)
    cols = np.asarray(cols)
    n_users = int(np.asarray(n_users).ravel()[0])
    t_mid_f = float(np.asarray(t_mid).ravel()[0])
    t_half_f = float(np.asarray(t_half).ravel()[0])
    n_items = X_batch.shape[1]
    assert X_batch.shape[0] == BATCH and n_items <= ITEMS_PAD

    X = sp.csr_matrix((vals.astype(np.float64), (rows, cols)),
                      shape=(n_users, n_items))
    B = (X.T @ X).toarray().astype(np.float32)

    key = ("nc", len(coeffs), t_mid_f, t_half_f, tuple(np.asarray(coeffs, np.float64)))
    nc = _cache.get(key)
    if nc is None:
        nc = _build(coeffs, t_mid_f, t_half_f)
        _cache[key] = nc

    in_maps = _prep_inputs(B, X_batch)
    res = bass_utils.run_bass_kernel_spmd(nc, in_maps, core_ids=list(range(N_CORES)))

    full = np.empty((ITEMS_PAD, BATCH), dtype=np.float32)
    for c in range(N_CORES):
        a = np.asarray(res.results[c]["acc_out"])      # [p, m, b]
        full[c * SLAB:(c + 1) * SLAB] = (
            a.transpose(1, 0, 2).reshape(SLAB, BATCH))
    return np.ascontiguousarray(full[:n_items].T)      # [BATCH, n_items] f32
